# revision 1
# baseline (speedup 1.0000x reference)
"""GraphSAGE-style encoder kernel for Trainium2 (8 NeuronCores).

Computes out = relu(W @ concat([F[nodes], mean(F[neigh_idx], 1)], 1).T)
for F [100000, 512] f32, W [512, 1024] f32, nodes [16384], neigh [16384, 32].

Sharding: data-parallel over the node batch B=16384 -> 2048 nodes/core; the
feature table and weight are replicated (the table is host-cast to bf16,
halving gather traffic; all device compute accumulates in f32 PSUM).

Per-core device pipeline (Bass/Tile, ANT dma_gather):
  - dma_gather uses int16 row indices, so table rows are addressed through
    4 base-offset classes (rows 0/32768/65536/98304); rows are gathered per
    (half-chunk of 256 nodes, class) with static per-(tile,class) caps, on
    4 SWDGE queues, with each sub-list sorted by row id for HBM locality.
  - Per-node sums (self row, sum of 32 neighbor rows) are recovered from
    the class-scattered gather positions with on-device-built selection
    matrices (iota + is_equal) contracted on the tensor engine into
    per-tile f32 PSUM accumulators [128 nodes, 512 feats].
  - PSUM -> SBUF (bf16) -> PE transposes build C^T [feat, node]; bf16
    matmuls against the host-prepped W^T (neighbor half pre-scaled by
    1/32) accumulate in f32 PSUM; fused ReLU on the scalar engine; f32 out.
"""

import sys

if "/opt/trn_rl_repo" not in sys.path:
    sys.path.insert(0, "/opt/trn_rl_repo")

import numpy as np
import ml_dtypes

N_TOTAL = 100000
FEAT = 512
EMBED = 512
B = 16384
NSAMP = 32
NCORES = 8
P = 128
BC = B // NCORES   # 2048 nodes/core
NT = BC // P       # 16 tiles of 128 nodes
NHC = NT // 2      # 8 half-chunks of 2 tiles

CLS_BASE = [0, 32768, 65536, 98304]
CLS_SIZE = [32768, 32768, 32768, N_TOTAL - 98304]
CAP = [1536, 1536, 1536, 256]          # positions per (tile, class)
CH = [c // P for c in CAP]             # chunks per (tile, class)
SUM_CH = sum(CH)
IDC_PER_T = 4 + SUM_CH                 # id columns per tile
CUM_CH = [0, CH[0], CH[0] + CH[1], CH[0] + CH[1] + CH[2]]
CALL_LEN = [2 * c for c in CAP]        # positions per (half-chunk, class)
IDX_TOT = NHC * sum(CALL_LEN)

_CACHE = {}


def build_nc():
    import concourse.bass as bass  # noqa: F401
    import concourse.mybir as mybir
    import concourse.tile as tile
    from concourse import bacc
    from concourse.masks import make_identity

    dt = mybir.dt

    nc = bacc.Bacc(
        "TRN2",
        target_bir_lowering=False,
        debug=False,
        enable_asserts=False,
        num_devices=NCORES,
        num_swdge_queues=4,
    )

    feat_d = nc.dram_tensor("feat", [N_TOTAL, FEAT], dt.bfloat16,
                            kind="ExternalInput").ap()
    w_t = nc.dram_tensor("w_t", [2 * FEAT, EMBED], dt.bfloat16,
                         kind="ExternalInput").ap()
    idx_d = nc.dram_tensor("idx", [P, IDX_TOT // 16], dt.int16,
                           kind="ExternalInput").ap()
    ids_d = nc.dram_tensor("ids", [P, NT * IDC_PER_T], dt.bfloat16,
                           kind="ExternalInput").ap()
    out_d = nc.dram_tensor("out", [EMBED, BC], dt.float32,
                           kind="ExternalOutput").ap()

    with tile.TileContext(nc) as tc:
        with (
            tc.tile_pool(name="const", bufs=1) as cpool,
            tc.tile_pool(name="gather", bufs=5) as gpool,
            tc.tile_pool(name="rbuf", bufs=2) as rpool,
            tc.tile_pool(name="snb", bufs=4) as spool,
            tc.tile_pool(name="ct", bufs=1) as ctpool,
            tc.tile_pool(name="ot", bufs=2) as opool,
            tc.tile_pool(name="psum_acc", bufs=3, space="PSUM") as papool,
            tc.tile_pool(name="psum_x", bufs=2, space="PSUM") as pxpool,
        ):
            ident = cpool.tile([P, P], dt.bfloat16, tag="ident", name="ident")
            make_identity(nc, ident[:])
            iota_t = cpool.tile([P, max(CH), P], dt.bfloat16, tag="iota",
                                name="iota_t")
            nc.gpsimd.iota(iota_t[:], pattern=[[0, max(CH)], [1, P]], base=0,
                           channel_multiplier=0,
                           allow_small_or_imprecise_dtypes=True)

            wt = []
            for k in range(8):
                t_ = cpool.tile([P, EMBED], dt.bfloat16, tag=f"wt{k}",
                                name=f"wt{k}")
                nc.sync.dma_start(out=t_[:], in_=w_t[k * P:(k + 1) * P, :])
                wt.append(t_)

            idxs = cpool.tile([P, IDX_TOT // 16], dt.int16, tag="idxs",
                              name="idxs")
            nc.sync.dma_start(out=idxs[:], in_=idx_d[:, :])
            ids = cpool.tile([P, NT * IDC_PER_T], dt.bfloat16, tag="ids",
                             name="ids")
            nc.sync.dma_start(out=ids[:], in_=ids_d[:, :])

            ct = [[ctpool.tile([P, 4 * P], dt.bfloat16, tag=f"ct{n}_{k}",
                               name=f"ct{n}_{k}")
                   for k in range(8)] for n in range(NT // 4)]

            idx_off = 0
            for hc in range(NHC):
                t0 = 2 * hc
                G = []
                for c in range(4):
                    g = gpool.tile([P, 2 * CH[c], FEAT], dt.bfloat16,
                                   tag="G", name=f"g{hc}_{c}")
                    nc.gpsimd.dma_gather(
                        g[:],
                        feat_d[CLS_BASE[c]:CLS_BASE[c] + CLS_SIZE[c], :],
                        idxs[:, idx_off // 16:(idx_off + CALL_LEN[c]) // 16],
                        CALL_LEN[c], CALL_LEN[c], FEAT,
                        single_packet=False, queue_num=c)
                    idx_off += CALL_LEN[c]
                    G.append(g)

                ps = {}
                pn = {}
                for t in (t0, t0 + 1):
                    ps[t] = papool.tile([P, 4 * P], dt.float32, tag="ps",
                                        name=f"ps{t}")
                    pn[t] = papool.tile([P, 4 * P], dt.float32, tag="pn",
                                        name=f"pn{t}")

                for c in range(4):
                    for sub, t in enumerate((t0, t0 + 1)):
                        idbase = t * IDC_PER_T
                        rb = rpool.tile([P, CH[c] * P], dt.bfloat16, tag="rb",
                                        name=f"rb{hc}_{c}_{sub}")
                        nc.vector.tensor_tensor(
                            out=rb[:].rearrange("p (c q) -> p c q", q=P),
                            in0=ids[:, idbase + 4 + CUM_CH[c]:
                                    idbase + 4 + CUM_CH[c] + CH[c]]
                                .to_broadcast([P, CH[c], P]),
                            in1=iota_t[:, :CH[c], :],
                            op=mybir.AluOpType.is_equal)
                        rs = rpool.tile([P, P], dt.bfloat16, tag="rs",
                                        name=f"rs{hc}_{c}_{sub}")
                        nc.vector.tensor_tensor(
                            out=rs[:],
                            in0=ids[:, idbase + c:idbase + c + 1]
                                .to_broadcast([P, P]),
                            in1=iota_t[:, 0, :],
                            op=mybir.AluOpType.is_equal)

                        slot0 = sub * CH[c]
                        for k in range(CH[c]):
                            nc.tensor.matmul(
                                out=pn[t][:],
                                lhsT=rb[:, k * P:(k + 1) * P],
                                rhs=G[c][:, slot0 + k, :],
                                start=(c == 0 and k == 0),
                                stop=(c == 3 and k == CH[3] - 1))
                        nc.tensor.matmul(
                            out=ps[t][:],
                            lhsT=rs[:],
                            rhs=G[c][:, slot0, :],
                            start=(c == 0), stop=(c == 3))

                for t in (t0, t0 + 1):
                    n = t // 4
                    col = (t % 4) * P
                    ssb = spool.tile([P, FEAT], dt.bfloat16, tag="ssb",
                                     name=f"ssb{t}")
                    nc.vector.tensor_copy(out=ssb[:], in_=ps[t][:])
                    nsb = spool.tile([P, FEAT], dt.bfloat16, tag="nsb",
                                     name=f"nsb{t}")
                    nc.vector.tensor_copy(out=nsb[:], in_=pn[t][:])
                    for cc in range(4):
                        pt1 = pxpool.tile([P, P], dt.bfloat16, tag="px",
                                          name=f"pt{t}_{cc}")
                        nc.tensor.transpose(out=pt1[:],
                                            in_=ssb[:, cc * P:(cc + 1) * P],
                                            identity=ident[:])
                        nc.vector.tensor_copy(out=ct[n][cc][:, col:col + P],
                                              in_=pt1[:])
                        pt2 = pxpool.tile([P, P], dt.bfloat16, tag="px",
                                          name=f"pt{t}_n{cc}")
                        nc.tensor.transpose(out=pt2[:],
                                            in_=nsb[:, cc * P:(cc + 1) * P],
                                            identity=ident[:])
                        nc.vector.tensor_copy(
                            out=ct[n][4 + cc][:, col:col + P], in_=pt2[:])

                if hc % 2 == 1:
                    n = hc // 2
                    for m in range(4):
                        pm = pxpool.tile([P, 4 * P], dt.float32, tag="px",
                                         name=f"pm{n}_{m}")
                        for k in range(8):
                            nc.tensor.matmul(
                                out=pm[:],
                                lhsT=wt[k][:, m * P:(m + 1) * P],
                                rhs=ct[n][k][:],
                                start=(k == 0),
                                stop=(k == 7))
                        ot = opool.tile([P, 4 * P], dt.float32, tag="ot",
                                        name=f"ot{n}_{m}")
                        nc.scalar.activation(
                            out=ot[:], in_=pm[:],
                            func=mybir.ActivationFunctionType.Relu)
                        nc.sync.dma_start(
                            out=out_d[m * P:(m + 1) * P,
                                      n * 4 * P:(n + 1) * 4 * P],
                            in_=ot[:])

            assert idx_off == IDX_TOT

    nc.compile()
    return nc


def get_nc():
    if "nc" not in _CACHE:
        _CACHE["nc"] = build_nc()
    return _CACHE["nc"]


def _classify(r):
    return np.searchsorted(np.asarray(CLS_BASE[1:]), r, side="right")


def _wrap_idxs(idx, pad_to):
    """dma_gather idx layout: int16, value [ch, i] = idx[i*16+ch], wrapped
    into 16 partitions and replicated across the 8 groups of 16."""
    idx = np.asarray(idx, dtype=np.int64)
    n = len(idx)
    assert n <= pad_to, (n, pad_to)
    idx = np.concatenate([idx, np.zeros(pad_to - n, np.int64)])
    assert idx.max() <= 32767 and idx.min() >= 0
    wrapped = idx.astype(np.int16).reshape(pad_to // 16, 16).T
    return np.tile(wrapped, (8, 1))


def prep_core(nodes_c, neigh_c):
    cls_self = _classify(nodes_c)
    cls_neigh = _classify(neigh_c)

    idx_all = []
    ids = np.full((P, NT * IDC_PER_T), 255.0, np.float32)

    for hc in range(NHC):
        for c in range(4):
            for t in (2 * hc, 2 * hc + 1):
                lo = t * P
                nd = nodes_c[lo:lo + P]
                ng = neigh_c[lo:lo + P]
                jj_self = np.nonzero(cls_self[lo:lo + P] == c)[0]
                jn, sn = np.nonzero(cls_neigh[lo:lo + P] == c)
                o = np.argsort(nd[jj_self], kind="stable")
                jj_self = jj_self[o]
                o = np.argsort(ng[jn, sn], kind="stable")
                jn, sn = jn[o], sn[o]
                n_s, n_n = len(jj_self), len(jn)
                assert n_s <= P, "self rows of one class exceed a chunk"
                assert n_s + n_n <= CAP[c], (
                    f"tile {t} class {c}: {n_s}+{n_n} > {CAP[c]}")
                rows = np.concatenate([
                    nd[jj_self] - CLS_BASE[c],
                    ng[jn, sn] - CLS_BASE[c],
                    np.zeros(CAP[c] - n_s - n_n, np.int64),
                ])
                idx_all.append(rows)
                idbase = t * IDC_PER_T
                col = np.full(P, 255.0, np.float32)
                col[:n_s] = jj_self
                ids[:, idbase + c] = col
                q = np.full(CAP[c], 255.0, np.float32)
                q[n_s:n_s + n_n] = jn
                ids[:, idbase + 4 + CUM_CH[c]:
                    idbase + 4 + CUM_CH[c] + CH[c]] = q.reshape(CH[c], P).T
    idx_flat = np.concatenate(idx_all)
    assert len(idx_flat) == IDX_TOT
    idx_arr = np.ascontiguousarray(_wrap_idxs(idx_flat, IDX_TOT))
    return idx_arr, ids.astype(ml_dtypes.bfloat16)


def make_in_maps(features, weight, nodes, neigh_idx):
    features = np.asarray(features, dtype=np.float32)
    weight = np.asarray(weight, dtype=np.float32)
    nodes = np.asarray(nodes).astype(np.int64)
    neigh_idx = np.asarray(neigh_idx).astype(np.int64)

    feat_bf16 = features.astype(ml_dtypes.bfloat16)
    wt = weight.T.copy()
    wt[FEAT:, :] *= (1.0 / NSAMP)   # fold the neighbor mean into W
    wt_bf16 = np.ascontiguousarray(wt.astype(ml_dtypes.bfloat16))

    in_maps = []
    for c in range(NCORES):
        nd = nodes[c * BC:(c + 1) * BC]
        ng = neigh_idx[c * BC:(c + 1) * BC]
        idx_arr, ids = prep_core(nd, ng)
        in_maps.append({
            "feat": feat_bf16,
            "w_t": wt_bf16,
            "idx": idx_arr,
            "ids": ids,
        })
    return in_maps


def kernel(features, weight, nodes, neigh_idx):
    from concourse import bass_utils

    nc = get_nc()
    in_maps = make_in_maps(features, weight, nodes, neigh_idx)
    res = bass_utils.run_bass_kernel_spmd(
        nc, in_maps, core_ids=list(range(NCORES)), trace=False)
    out = np.concatenate([np.asarray(r["out"]) for r in res.results], axis=1)
    return out



# revision 2
# speedup vs baseline: 9.0509x; 9.0509x over previous
"""GraphSAGE-style encoder kernel for Trainium2 (8 NeuronCores).

out = relu(W @ concat([F[nodes], mean(F[neigh_idx], 1)], 1).T)
F [100000, 512] f32, W [512, 1024] f32, nodes [16384], neigh [16384, 32].

Data-parallel over B across 8 cores (2048 nodes/core); table + weight
replicated. The gather dominates: HW cost is ~5-6ns per gathered row
(descriptor-rate / random-access bound), so v5 attacks descriptor count:

  - fp8-e4m3 table copy for the neighbor gather (~17%/desc cheaper, 512B
    rows), bf16 self rows (accuracy), DoubleRow fp8 aggregation.
  - Per-(tile,class) gather sections are sized to the MAX ACTUAL count over
    the 8 cores (rounded up to 16), not to a 5-sigma static cap: padding
    drops from ~21% of descriptors to ~2%. Section sizes are baked into the
    program at build time from the real inputs (all cores share one static
    layout); a different input set just triggers a rebuild.
  - Self rows are gathered in 4 whole-core calls (one per int16 class).
  - Output is written bf16 and upcast to f32 on the host (halves out DMA).

Aggregation as before: on-device selection matrices (iota + is_equal, fp8
0/1 exact), f32 PSUM accumulators, PE transposes to C^T, bf16 matmul vs
host-prepped W^T (neighbor half pre-scaled by 1/32), fused ReLU.
"""

import sys

if "/opt/trn_rl_repo" not in sys.path:
    sys.path.insert(0, "/opt/trn_rl_repo")

import numpy as np
import ml_dtypes

N_TOTAL = 100000
FEAT = 512
EMBED = 512
B = 16384
NSAMP = 32
NCORES = 8
P = 128
BC = B // NCORES
NT = BC // P
NHC = NT // 2

CLS_BASE = [0, 32768, 65536, 98304]
CLS_SIZE = [32768, 32768, 32768, N_TOTAL - 98304]
CAP_N = [1536, 1536, 1536, 256]    # hard upper bounds (layout sanity only)

_CACHE = {}


def _r16(x):
    return (int(x) + 15) // 16 * 16


def build_nc(sec_n, sec_s, reps=1):
    """sec_n[t][c]: neighbor idx-section length for (tile t, class c);
    sec_s[t][c]: self section length. All multiples of 16."""
    import concourse.bass as bass  # noqa: F401
    import concourse.mybir as mybir
    import concourse.tile as tile
    from concourse import bacc
    from concourse.masks import make_identity

    dt = mybir.dt

    ch_n = [[(sec_n[t][c] + P - 1) // P for c in range(4)] for t in range(NT)]
    max_ch = max(max(r) for r in ch_n)
    call_n = [[sec_n[2 * hc][c] + sec_n[2 * hc + 1][c] for c in range(4)]
              for hc in range(NHC)]
    call_s = [sum(sec_s[hc][c] for hc in range(NHC)) for c in range(4)]
    gch_s = [(call_s[c] + P - 1) // P for c in range(4)]
    max_sch = max(sec_s[hc][c] // P for hc in range(NHC) for c in range(4))
    hc_len = [sum(call_n[hc]) for hc in range(NHC)]
    idx_tot = sum(call_s) + sum(hc_len)
    # chunk offset of each pair-section within gself[c]
    soff = [[0] * 4 for _ in range(NHC)]
    for c in range(4):
        off = 0
        for hc in range(NHC):
            soff[hc][c] = off
            off += sec_s[hc][c]

    nc = bacc.Bacc(
        "TRN2",
        target_bir_lowering=False,
        debug=False,
        enable_asserts=False,
        num_devices=NCORES,
        num_swdge_queues=4,
    )

    feat_d = nc.dram_tensor("feat", [N_TOTAL, FEAT], dt.bfloat16,
                            kind="ExternalInput").ap()
    featq_d = nc.dram_tensor("featq", [N_TOTAL, FEAT], dt.float8e4,
                             kind="ExternalInput").ap()
    w_t = nc.dram_tensor("w_t", [2 * FEAT, EMBED], dt.bfloat16,
                         kind="ExternalInput").ap()
    idx_d = nc.dram_tensor("idx", [P, idx_tot // 16], dt.int16,
                           kind="ExternalInput").ap()
    ids_d = nc.dram_tensor("ids", [P, NT * (4 * max_sch + 4 * max_ch)],
                           dt.bfloat16, kind="ExternalInput").ap()
    out_d = nc.dram_tensor("out", [EMBED, BC], dt.bfloat16,
                           kind="ExternalOutput").ap()
    idc_per_t = 4 * max_sch + 4 * max_ch

    with tile.TileContext(nc) as tc:
        with (
            tc.tile_pool(name="const", bufs=1) as cpool,
            tc.tile_pool(name="gather", bufs=2) as gpool,
            tc.tile_pool(name="rbuf", bufs=2) as rpool,
            tc.tile_pool(name="snb", bufs=4) as spool,
            tc.tile_pool(name="ct", bufs=1) as ctpool,
            tc.tile_pool(name="ot", bufs=2) as opool,
            tc.tile_pool(name="psum_acc", bufs=3, space="PSUM") as papool,
            tc.tile_pool(name="psum_x", bufs=2, space="PSUM") as pxpool,
        ):
            ident = cpool.tile([P, P], dt.bfloat16, tag="ident", name="ident")
            make_identity(nc, ident[:])
            iota_t = cpool.tile([P, max_ch, P], dt.bfloat16, tag="iota",
                                name="iota_t")
            nc.gpsimd.iota(iota_t[:], pattern=[[0, max_ch], [1, P]], base=0,
                           channel_multiplier=0,
                           allow_small_or_imprecise_dtypes=True)

            wt = []
            for k in range(8):
                t_ = cpool.tile([P, EMBED], dt.bfloat16, tag=f"wt{k}",
                                name=f"wt{k}")
                nc.sync.dma_start(out=t_[:], in_=w_t[k * P:(k + 1) * P, :])
                wt.append(t_)

            sidx = cpool.tile([P, sum(call_s) // 16], dt.int16, tag="sidx",
                              name="sidx")
            nc.sync.dma_start(out=sidx[:], in_=idx_d[:, :sum(call_s) // 16])
            ids = cpool.tile([P, NT * idc_per_t], dt.bfloat16, tag="ids",
                             name="ids")
            nc.sync.dma_start(out=ids[:], in_=ids_d[:, :])

            ct = [[ctpool.tile([P, 4 * P], dt.bfloat16, tag=f"ct{n}_{k}",
                               name=f"ct{n}_{k}")
                   for k in range(8)] for n in range(NT // 4)]

            GS = [cpool.tile([P, gch_s[c], FEAT], dt.bfloat16, tag=f"S{c}",
                             name=f"s_{c}") for c in range(4)]

            for _rep in range(reps):
                off = 0
                for c in range(4):
                    nc.gpsimd.dma_gather(
                        GS[c][:],
                        feat_d[CLS_BASE[c]:CLS_BASE[c] + CLS_SIZE[c], :],
                        sidx[:, off // 16:(off + call_s[c]) // 16],
                        call_s[c], call_s[c], FEAT,
                        single_packet=False, queue_num=c)
                    off += call_s[c]
                idx_off = sum(call_s)
                for hc in range(NHC):
                    t0 = 2 * hc
                    nidx = gpool.tile([P, max(hc_len) // 16], dt.int16,
                                      tag="nidx", name=f"nidx{hc}")
                    nc.sync.dma_start(
                        out=nidx[:, :hc_len[hc] // 16],
                        in_=idx_d[:, idx_off // 16:
                                  (idx_off + hc_len[hc]) // 16])
                    noff = 0
                    G = []
                    for c in range(4):
                        nch = (call_n[hc][c] + P - 1) // P
                        g = gpool.tile([P, 2 * CAP_N[c] // P, FEAT],
                                       dt.float8e4, tag=f"G{c}",
                                       name=f"g{hc}_{c}")
                        nc.gpsimd.dma_gather(
                            g[:, :nch, :],
                            featq_d[CLS_BASE[c]:CLS_BASE[c] + CLS_SIZE[c], :],
                            nidx[:, noff // 16:(noff + call_n[hc][c]) // 16],
                            call_n[hc][c], call_n[hc][c], FEAT,
                            single_packet=False, queue_num=c)
                        idx_off += call_n[hc][c]
                        noff += call_n[hc][c]
                        G.append(g)

                    ps = {}
                    pn = {}
                    for t in (t0, t0 + 1):
                        ps[t] = papool.tile([P, 4 * P], dt.float32, tag="ps",
                                            name=f"ps{t}")
                        pn[t] = papool.tile([P, 4 * P], dt.float32, tag="pn",
                                            name=f"pn{t}")

                    # last (class, k) neighbor matmul per tile for stop flags
                    for sub, t in enumerate((t0, t0 + 1)):
                        idbase = t * idc_per_t
                        nmm = []   # (c, chunk_in_call, kth-of-tile)
                        for c in range(4):
                            base_pos = sub * sec_n[t0][c] if sub else 0
                            nch_t = (sec_n[t][c] + P - 1) // P
                            # section start is 16-aligned but not 128-aligned;
                            # sections are laid out back-to-back per call.
                            # We require 128 alignment for chunk mapping:
                            # sec_n entries are rounded to 128 by the host.
                            assert sec_n[t][c] % P == 0
                            first_ch = base_pos // P
                            for k in range(nch_t):
                                nmm.append((c, first_ch + k, k))
                        rbs = {}
                        for c in range(4):
                            nch_t = (sec_n[t][c] + P - 1) // P
                            if nch_t == 0:
                                continue
                            rb = rpool.tile([P, max_ch * P], dt.float8e4,
                                            tag="rb", name=f"rb{hc}_{c}_{sub}")
                            nc.vector.tensor_tensor(
                                out=rb[:, :nch_t * P]
                                    .rearrange("p (c q) -> p c q", q=P),
                                in0=ids[:, idbase + 4 * max_sch + c * max_ch:
                                        idbase + 4 * max_sch + c * max_ch
                                        + nch_t]
                                    .to_broadcast([P, nch_t, P]),
                                in1=iota_t[:, :nch_t, :],
                                op=mybir.AluOpType.is_equal)
                            rbs[c] = rb
                        # DoubleRow pairs within each class section
                        n_items = len(nmm)
                        i = 0
                        first = True
                        while i < n_items:
                            c, ch_i, k_i = nmm[i]
                            pair = (i + 1 < n_items and nmm[i + 1][0] == c
                                    and nmm[i + 1][1] == ch_i + 1)
                            is_last = (i + (2 if pair else 1)) >= n_items
                            if pair:
                                nc.tensor.matmul(
                                    out=pn[t][:],
                                    lhsT=rbs[c][:, k_i * P:(k_i + 2) * P]
                                        .rearrange("p (c q) -> p c q", q=P),
                                    rhs=G[c][:, ch_i:ch_i + 2, :],
                                    perf_mode=mybir.MatmulPerfMode.DoubleRow,
                                    start=first, stop=is_last)
                                i += 2
                            else:
                                nc.tensor.matmul(
                                    out=pn[t][:],
                                    lhsT=rbs[c][:, k_i * P:(k_i + 1) * P],
                                    rhs=G[c][:, ch_i, :],
                                    start=first, stop=is_last)
                                i += 1
                            first = False

                        # self: pair (hc) section per class; tile's rows sit
                        # somewhere inside, ids mask the other tile's rows
                        smm = []
                        for c in range(4):
                            assert sec_s[hc][c] % P == 0
                            for k in range(sec_s[hc][c] // P):
                                smm.append((c, soff[hc][c] // P + k, k))
                        for j, (c, ch_i, k_i) in enumerate(smm):
                            rs = rpool.tile([P, P], dt.bfloat16, tag="rs",
                                            name=f"rs{hc}_{c}_{sub}_{k_i}")
                            nc.vector.tensor_tensor(
                                out=rs[:],
                                in0=ids[:, idbase + c * max_sch + k_i:
                                        idbase + c * max_sch + k_i + 1]
                                    .to_broadcast([P, P]),
                                in1=iota_t[:, 0, :],
                                op=mybir.AluOpType.is_equal)
                            nc.tensor.matmul(
                                out=ps[t][:],
                                lhsT=rs[:],
                                rhs=GS[c][:, ch_i, :],
                                start=(j == 0), stop=(j == len(smm) - 1))

                    for t in (t0, t0 + 1):
                        n = t // 4
                        col = (t % 4) * P
                        ssb = spool.tile([P, FEAT], dt.bfloat16, tag="ssb",
                                         name=f"ssb{t}")
                        nc.vector.tensor_copy(out=ssb[:], in_=ps[t][:])
                        nsb = spool.tile([P, FEAT], dt.bfloat16, tag="nsb",
                                         name=f"nsb{t}")
                        nc.vector.tensor_copy(out=nsb[:], in_=pn[t][:])
                        for cc in range(4):
                            pt1 = pxpool.tile([P, P], dt.bfloat16, tag="px",
                                              name=f"pt{t}_{cc}")
                            nc.tensor.transpose(
                                out=pt1[:], in_=ssb[:, cc * P:(cc + 1) * P],
                                identity=ident[:])
                            nc.vector.tensor_copy(
                                out=ct[n][cc][:, col:col + P], in_=pt1[:])
                            pt2 = pxpool.tile([P, P], dt.bfloat16, tag="px",
                                              name=f"pt{t}_n{cc}")
                            nc.tensor.transpose(
                                out=pt2[:], in_=nsb[:, cc * P:(cc + 1) * P],
                                identity=ident[:])
                            nc.vector.tensor_copy(
                                out=ct[n][4 + cc][:, col:col + P], in_=pt2[:])

                    if hc % 2 == 1:
                        n = hc // 2
                        for m in range(4):
                            pm = pxpool.tile([P, 4 * P], dt.float32, tag="px",
                                             name=f"pm{n}_{m}")
                            for k in range(8):
                                nc.tensor.matmul(
                                    out=pm[:],
                                    lhsT=wt[k][:, m * P:(m + 1) * P],
                                    rhs=ct[n][k][:],
                                    start=(k == 0), stop=(k == 7))
                            ot = opool.tile([P, 4 * P], dt.bfloat16, tag="ot",
                                            name=f"ot{n}_{m}")
                            nc.scalar.activation(
                                out=ot[:], in_=pm[:],
                                func=mybir.ActivationFunctionType.Relu)
                            nc.sync.dma_start(
                                out=out_d[m * P:(m + 1) * P,
                                          n * 4 * P:(n + 1) * 4 * P],
                                in_=ot[:])

                assert idx_off == idx_tot

    nc.compile()
    return nc


def _classify(r):
    return np.searchsorted(np.asarray(CLS_BASE[1:]), r, side="right")


def _wrap_idxs(idx, pad_to):
    idx = np.asarray(idx, dtype=np.int64)
    n = len(idx)
    assert n <= pad_to, (n, pad_to)
    idx = np.concatenate([idx, np.zeros(pad_to - n, np.int64)])
    assert idx.max() <= 32767 and idx.min() >= 0
    wrapped = idx.astype(np.int16).reshape(pad_to // 16, 16).T
    return np.tile(wrapped, (8, 1))


def plan_sections(nodes, neigh_idx):
    """Neighbor sections: max-over-cores per-(tile,class) counts; self
    sections: max-over-cores per-(tile-pair,class). All 128-aligned."""
    nodes = np.asarray(nodes).astype(np.int64)
    neigh_idx = np.asarray(neigh_idx).astype(np.int64)
    sec_n = [[0] * 4 for _ in range(NT)]
    sec_s = [[0] * 4 for _ in range(NHC)]
    for core in range(NCORES):
        nd = nodes[core * BC:(core + 1) * BC]
        ng = neigh_idx[core * BC:(core + 1) * BC]
        cs = _classify(nd)
        cn = _classify(ng)
        for t in range(NT):
            lo = t * P
            for c in range(4):
                n_n = int((cn[lo:lo + P] == c).sum())
                sec_n[t][c] = max(sec_n[t][c], n_n)
        for hc in range(NHC):
            lo = 2 * hc * P
            for c in range(4):
                n_s = int((cs[lo:lo + 2 * P] == c).sum())
                sec_s[hc][c] = max(sec_s[hc][c], n_s)
    r128 = lambda x: (x + P - 1) // P * P
    sec_n = [[r128(sec_n[t][c]) for c in range(4)] for t in range(NT)]
    sec_s = [[r128(sec_s[hc][c]) for c in range(4)] for hc in range(NHC)]
    for t in range(NT):
        for c in range(4):
            assert sec_n[t][c] <= CAP_N[c], (t, c, sec_n[t][c])
    return sec_n, sec_s


def prep_core(nodes_c, neigh_c, sec_n, sec_s):
    cls_self = _classify(nodes_c)
    cls_neigh = _classify(neigh_c)
    ch_n = [[(sec_n[t][c] + P - 1) // P for c in range(4)] for t in range(NT)]
    max_ch = max(max(r) for r in ch_n)
    max_sch = max(sec_s[hc][c] // P for hc in range(NHC) for c in range(4))
    idc_per_t = 4 * max_sch + 4 * max_ch

    idx_all = []
    ids = np.full((P, NT * idc_per_t), 255.0, np.float32)

    # self streams: per class, per tile-pair section [t0 rows | t1 rows | pad]
    for c in range(4):
        for hc in range(NHC):
            t0 = 2 * hc
            secpos = []   # (tile, node_id_within_tile) per position
            rows = []
            for t in (t0, t0 + 1):
                lo = t * P
                nd = nodes_c[lo:lo + P]
                jj = np.nonzero(cls_self[lo:lo + P] == c)[0]
                o = np.argsort(nd[jj], kind="stable")
                jj = jj[o]
                rows.extend((nd[jj] - CLS_BASE[c]).tolist())
                secpos.extend((t, int(j)) for j in jj)
            sec = sec_s[hc][c]
            assert len(rows) <= sec, (hc, c, len(rows))
            rows.extend([0] * (sec - len(rows)))
            idx_all.append(np.asarray(rows, np.int64))
            # per-tile masked id planes over this section's chunks
            for t in (t0, t0 + 1):
                idbase = t * idc_per_t
                q = np.full(sec, 255.0, np.float32)
                for pos, (tt, j) in enumerate(secpos):
                    if tt == t:
                        q[pos] = j
                ids[:, idbase + c * max_sch:
                    idbase + c * max_sch + sec // P] =                     q.reshape(sec // P, P).T

    # neighbor streams per (hc, class): [t0 section, t1 section]
    for hc in range(NHC):
        for c in range(4):
            for t in (2 * hc, 2 * hc + 1):
                lo = t * P
                ng = neigh_c[lo:lo + P]
                jn, sn = np.nonzero(cls_neigh[lo:lo + P] == c)
                o = np.argsort(ng[jn, sn], kind="stable")
                jn, sn = jn[o], sn[o]
                n_n = len(jn)
                assert n_n <= sec_n[t][c], (t, c, n_n)
                rows = np.concatenate([
                    ng[jn, sn] - CLS_BASE[c],
                    np.zeros(sec_n[t][c] - n_n, np.int64),
                ])
                idx_all.append(rows)
                idbase = t * idc_per_t
                q = np.full(sec_n[t][c], 255.0, np.float32)
                q[:n_n] = jn
                ids[:, idbase + 4 * max_sch + c * max_ch:
                    idbase + 4 * max_sch + c * max_ch + ch_n[t][c]] = \
                    q.reshape(ch_n[t][c], P).T
    idx_flat = np.concatenate(idx_all)
    idx_arr = np.ascontiguousarray(_wrap_idxs(idx_flat, len(idx_flat)))
    return idx_arr, ids.astype(ml_dtypes.bfloat16)


def make_in_maps(features, weight, nodes, neigh_idx, sec_n, sec_s):
    from concourse import mybir

    features = np.asarray(features, dtype=np.float32)
    weight = np.asarray(weight, dtype=np.float32)
    nodes = np.asarray(nodes).astype(np.int64)
    neigh_idx = np.asarray(neigh_idx).astype(np.int64)

    feat_bf16 = features.astype(ml_dtypes.bfloat16)
    fp8_np = mybir.dt.np(mybir.dt.float8e4)
    feat_fp8 = features.astype(fp8_np)
    wt = weight.T.copy()
    wt[FEAT:, :] *= (1.0 / NSAMP)
    wt_bf16 = np.ascontiguousarray(wt.astype(ml_dtypes.bfloat16))

    in_maps = []
    for c in range(NCORES):
        nd = nodes[c * BC:(c + 1) * BC]
        ng = neigh_idx[c * BC:(c + 1) * BC]
        idx_arr, ids = prep_core(nd, ng, sec_n, sec_s)
        in_maps.append({
            "feat": feat_bf16,
            "featq": feat_fp8,
            "w_t": wt_bf16,
            "idx": idx_arr,
            "ids": ids,
        })
    return in_maps


def get_nc(sec_n, sec_s, reps=1):
    key = (tuple(map(tuple, sec_n)), tuple(map(tuple, sec_s)), reps)
    if key not in _CACHE:
        _CACHE[key] = build_nc(sec_n, sec_s, reps=reps)
    return _CACHE[key]


def kernel(features, weight, nodes, neigh_idx):
    from concourse import bass_utils

    sec_n, sec_s = plan_sections(nodes, neigh_idx)
    nc = get_nc(sec_n, sec_s)
    in_maps = make_in_maps(features, weight, nodes, neigh_idx, sec_n, sec_s)
    res = bass_utils.run_bass_kernel_spmd(
        nc, in_maps, core_ids=list(range(NCORES)), trace=False)
    out = np.concatenate(
        [np.asarray(r["out"]).astype(np.float32) for r in res.results],
        axis=1)
    return out


# revision 3
# speedup vs baseline: 10.0011x; 1.1050x over previous
"""GraphSAGE-style encoder kernel for Trainium2 (8 NeuronCores), v8 rotq.

out = relu(W @ concat([F[nodes], mean(F[neigh_idx], 1)], 1).T)
F [100000, 512] f32, W [512, 1024] f32, nodes [16384], neigh [16384, 32].

Data-parallel over B across 8 cores (2048 nodes/core); table + weight
replicated. The gather dominates: HW cost is ~5-6ns per gathered row
(descriptor-rate / random-access bound), so v5 attacks descriptor count:

  - fp8-e4m3 table copy for the neighbor gather (~17%/desc cheaper, 512B
    rows), bf16 self rows (accuracy), DoubleRow fp8 aggregation.
  - Per-(tile,class) gather sections are sized to the MAX ACTUAL count over
    the 8 cores (rounded up to 16), not to a 5-sigma static cap: padding
    drops from ~21% of descriptors to ~2%. Section sizes are baked into the
    program at build time from the real inputs (all cores share one static
    layout); a different input set just triggers a rebuild.
  - Self rows are gathered in 4 whole-core calls (one per int16 class).
  - Output is written bf16 and upcast to f32 on the host (halves out DMA).

Aggregation as before: on-device selection matrices (iota + is_equal, fp8
0/1 exact), f32 PSUM accumulators, PE transposes to C^T, bf16 matmul vs
host-prepped W^T (neighbor half pre-scaled by 1/32), fused ReLU.
"""

import sys

if "/opt/trn_rl_repo" not in sys.path:
    sys.path.insert(0, "/opt/trn_rl_repo")

import numpy as np
import ml_dtypes

N_TOTAL = 100000
FEAT = 512
EMBED = 512
B = 16384
NSAMP = 32
NCORES = 8
P = 128
BC = B // NCORES
NT = BC // P
NHC = NT // 2

CLS_BASE = [0, 32768, 65536, 98304]
CLS_SIZE = [32768, 32768, 32768, N_TOTAL - 98304]
CAP_N = [1536, 1536, 1536, 256]    # hard upper bounds (layout sanity only)

_CACHE = {}


def _r16(x):
    return (int(x) + 15) // 16 * 16


def build_nc(sec_n, sec_s, reps=1):
    """sec_n[t][c]: neighbor idx-section length for (tile t, class c);
    sec_s[t][c]: self section length. All multiples of 16."""
    import concourse.bass as bass  # noqa: F401
    import concourse.mybir as mybir
    import concourse.tile as tile
    from concourse import bacc
    from concourse.masks import make_identity

    dt = mybir.dt

    ch_n = [[(sec_n[t][c] + P - 1) // P for c in range(4)] for t in range(NT)]
    max_ch = max(max(r) for r in ch_n)
    call_n = [[sec_n[2 * hc][c] + sec_n[2 * hc + 1][c] for c in range(4)]
              for hc in range(NHC)]
    call_s = [sum(sec_s[hc][c] for hc in range(NHC)) for c in range(4)]
    gch_s = [(call_s[c] + P - 1) // P for c in range(4)]
    max_sch = max(sec_s[hc][c] // P for hc in range(NHC) for c in range(4))
    hc_len = [sum(call_n[hc]) for hc in range(NHC)]
    idx_tot = sum(call_s) + sum(hc_len)
    # chunk offset of each pair-section within gself[c]
    soff = [[0] * 4 for _ in range(NHC)]
    for c in range(4):
        off = 0
        for hc in range(NHC):
            soff[hc][c] = off
            off += sec_s[hc][c]

    nc = bacc.Bacc(
        "TRN2",
        target_bir_lowering=False,
        debug=False,
        enable_asserts=False,
        num_devices=NCORES,
        num_swdge_queues=4,
    )

    feat_d = nc.dram_tensor("feat", [N_TOTAL, FEAT], dt.bfloat16,
                            kind="ExternalInput").ap()
    featq_d = nc.dram_tensor("featq", [N_TOTAL, FEAT], dt.float8e4,
                             kind="ExternalInput").ap()
    w_t = nc.dram_tensor("w_t", [2 * FEAT, EMBED], dt.bfloat16,
                         kind="ExternalInput").ap()
    idx_d = nc.dram_tensor("idx", [P, idx_tot // 16], dt.int16,
                           kind="ExternalInput").ap()
    ids_d = nc.dram_tensor("ids", [P, NT * (4 * max_sch + 4 * max_ch)],
                           dt.bfloat16, kind="ExternalInput").ap()
    out_d = nc.dram_tensor("out", [EMBED, BC], dt.bfloat16,
                           kind="ExternalOutput").ap()
    idc_per_t = 4 * max_sch + 4 * max_ch

    with tile.TileContext(nc) as tc:
        with (
            tc.tile_pool(name="const", bufs=1) as cpool,
            tc.tile_pool(name="gather", bufs=2) as gpool,
            tc.tile_pool(name="rbuf", bufs=2) as rpool,
            tc.tile_pool(name="snb", bufs=4) as spool,
            tc.tile_pool(name="ct", bufs=1) as ctpool,
            tc.tile_pool(name="ot", bufs=2) as opool,
            tc.tile_pool(name="psum_acc", bufs=3, space="PSUM") as papool,
            tc.tile_pool(name="psum_x", bufs=2, space="PSUM") as pxpool,
        ):
            ident = cpool.tile([P, P], dt.bfloat16, tag="ident", name="ident")
            make_identity(nc, ident[:])
            iota_t = cpool.tile([P, max_ch, P], dt.bfloat16, tag="iota",
                                name="iota_t")
            nc.gpsimd.iota(iota_t[:], pattern=[[0, max_ch], [1, P]], base=0,
                           channel_multiplier=0,
                           allow_small_or_imprecise_dtypes=True)

            wt = []
            for k in range(8):
                t_ = cpool.tile([P, EMBED], dt.bfloat16, tag=f"wt{k}",
                                name=f"wt{k}")
                nc.sync.dma_start(out=t_[:], in_=w_t[k * P:(k + 1) * P, :])
                wt.append(t_)

            sidx = cpool.tile([P, sum(call_s) // 16], dt.int16, tag="sidx",
                              name="sidx")
            nc.sync.dma_start(out=sidx[:], in_=idx_d[:, :sum(call_s) // 16])
            ids = cpool.tile([P, NT * idc_per_t], dt.bfloat16, tag="ids",
                             name="ids")
            nc.sync.dma_start(out=ids[:], in_=ids_d[:, :])

            ct = [[ctpool.tile([P, 4 * P], dt.bfloat16, tag=f"ct{n}_{k}",
                               name=f"ct{n}_{k}")
                   for k in range(8)] for n in range(NT // 4)]

            GS = [cpool.tile([P, gch_s[c], FEAT], dt.bfloat16, tag=f"S{c}",
                             name=f"s_{c}") for c in range(4)]

            for _rep in range(reps):
                off = 0
                for c in range(4):
                    nc.gpsimd.dma_gather(
                        GS[c][:],
                        feat_d[CLS_BASE[c]:CLS_BASE[c] + CLS_SIZE[c], :],
                        sidx[:, off // 16:(off + call_s[c]) // 16],
                        call_s[c], call_s[c], FEAT,
                        single_packet=False, queue_num=c)
                    off += call_s[c]
                idx_off = sum(call_s)
                for hc in range(NHC):
                    t0 = 2 * hc
                    nidx = gpool.tile([P, max(hc_len) // 16], dt.int16,
                                      tag="nidx", name=f"nidx{hc}")
                    nc.sync.dma_start(
                        out=nidx[:, :hc_len[hc] // 16],
                        in_=idx_d[:, idx_off // 16:
                                  (idx_off + hc_len[hc]) // 16])
                    noff = 0
                    G = []
                    for c in range(4):
                        nch = (call_n[hc][c] + P - 1) // P
                        g = gpool.tile([P, 2 * CAP_N[c] // P, FEAT],
                                       dt.float8e4, tag=f"G{c}",
                                       name=f"g{hc}_{c}")
                        nc.gpsimd.dma_gather(
                            g[:, :nch, :],
                            featq_d[CLS_BASE[c]:CLS_BASE[c] + CLS_SIZE[c], :],
                            nidx[:, noff // 16:(noff + call_n[hc][c]) // 16],
                            call_n[hc][c], call_n[hc][c], FEAT,
                            single_packet=False, queue_num=(c + hc) % 4)
                        idx_off += call_n[hc][c]
                        noff += call_n[hc][c]
                        G.append(g)

                    ps = {}
                    pn = {}
                    for t in (t0, t0 + 1):
                        ps[t] = papool.tile([P, 4 * P], dt.float32, tag="ps",
                                            name=f"ps{t}")
                        pn[t] = papool.tile([P, 4 * P], dt.float32, tag="pn",
                                            name=f"pn{t}")

                    # last (class, k) neighbor matmul per tile for stop flags
                    for sub, t in enumerate((t0, t0 + 1)):
                        idbase = t * idc_per_t
                        nmm = []   # (c, chunk_in_call, kth-of-tile)
                        for c in range(4):
                            base_pos = sub * sec_n[t0][c] if sub else 0
                            nch_t = (sec_n[t][c] + P - 1) // P
                            # section start is 16-aligned but not 128-aligned;
                            # sections are laid out back-to-back per call.
                            # We require 128 alignment for chunk mapping:
                            # sec_n entries are rounded to 128 by the host.
                            assert sec_n[t][c] % P == 0
                            first_ch = base_pos // P
                            for k in range(nch_t):
                                nmm.append((c, first_ch + k, k))
                        rbs = {}
                        for c in range(4):
                            nch_t = (sec_n[t][c] + P - 1) // P
                            if nch_t == 0:
                                continue
                            rb = rpool.tile([P, max_ch * P], dt.float8e4,
                                            tag="rb", name=f"rb{hc}_{c}_{sub}")
                            nc.vector.tensor_tensor(
                                out=rb[:, :nch_t * P]
                                    .rearrange("p (c q) -> p c q", q=P),
                                in0=ids[:, idbase + 4 * max_sch + c * max_ch:
                                        idbase + 4 * max_sch + c * max_ch
                                        + nch_t]
                                    .to_broadcast([P, nch_t, P]),
                                in1=iota_t[:, :nch_t, :],
                                op=mybir.AluOpType.is_equal)
                            rbs[c] = rb
                        # DoubleRow pairs within each class section
                        n_items = len(nmm)
                        i = 0
                        first = True
                        while i < n_items:
                            c, ch_i, k_i = nmm[i]
                            pair = (i + 1 < n_items and nmm[i + 1][0] == c
                                    and nmm[i + 1][1] == ch_i + 1)
                            is_last = (i + (2 if pair else 1)) >= n_items
                            if pair:
                                nc.tensor.matmul(
                                    out=pn[t][:],
                                    lhsT=rbs[c][:, k_i * P:(k_i + 2) * P]
                                        .rearrange("p (c q) -> p c q", q=P),
                                    rhs=G[c][:, ch_i:ch_i + 2, :],
                                    perf_mode=mybir.MatmulPerfMode.DoubleRow,
                                    start=first, stop=is_last)
                                i += 2
                            else:
                                nc.tensor.matmul(
                                    out=pn[t][:],
                                    lhsT=rbs[c][:, k_i * P:(k_i + 1) * P],
                                    rhs=G[c][:, ch_i, :],
                                    start=first, stop=is_last)
                                i += 1
                            first = False

                        # self: pair (hc) section per class; tile's rows sit
                        # somewhere inside, ids mask the other tile's rows
                        smm = []
                        for c in range(4):
                            assert sec_s[hc][c] % P == 0
                            for k in range(sec_s[hc][c] // P):
                                smm.append((c, soff[hc][c] // P + k, k))
                        for j, (c, ch_i, k_i) in enumerate(smm):
                            rs = rpool.tile([P, P], dt.bfloat16, tag="rs",
                                            name=f"rs{hc}_{c}_{sub}_{k_i}")
                            nc.vector.tensor_tensor(
                                out=rs[:],
                                in0=ids[:, idbase + c * max_sch + k_i:
                                        idbase + c * max_sch + k_i + 1]
                                    .to_broadcast([P, P]),
                                in1=iota_t[:, 0, :],
                                op=mybir.AluOpType.is_equal)
                            nc.tensor.matmul(
                                out=ps[t][:],
                                lhsT=rs[:],
                                rhs=GS[c][:, ch_i, :],
                                start=(j == 0), stop=(j == len(smm) - 1))

                    for t in (t0, t0 + 1):
                        n = t // 4
                        col = (t % 4) * P
                        ssb = spool.tile([P, FEAT], dt.bfloat16, tag="ssb",
                                         name=f"ssb{t}")
                        nc.vector.tensor_copy(out=ssb[:], in_=ps[t][:])
                        nsb = spool.tile([P, FEAT], dt.bfloat16, tag="nsb",
                                         name=f"nsb{t}")
                        nc.vector.tensor_copy(out=nsb[:], in_=pn[t][:])
                        for cc in range(4):
                            pt1 = pxpool.tile([P, P], dt.bfloat16, tag="px",
                                              name=f"pt{t}_{cc}")
                            nc.tensor.transpose(
                                out=pt1[:], in_=ssb[:, cc * P:(cc + 1) * P],
                                identity=ident[:])
                            nc.vector.tensor_copy(
                                out=ct[n][cc][:, col:col + P], in_=pt1[:])
                            pt2 = pxpool.tile([P, P], dt.bfloat16, tag="px",
                                              name=f"pt{t}_n{cc}")
                            nc.tensor.transpose(
                                out=pt2[:], in_=nsb[:, cc * P:(cc + 1) * P],
                                identity=ident[:])
                            nc.vector.tensor_copy(
                                out=ct[n][4 + cc][:, col:col + P], in_=pt2[:])

                    if hc % 2 == 1:
                        n = hc // 2
                        for m in range(4):
                            pm = pxpool.tile([P, 4 * P], dt.float32, tag="px",
                                             name=f"pm{n}_{m}")
                            for k in range(8):
                                nc.tensor.matmul(
                                    out=pm[:],
                                    lhsT=wt[k][:, m * P:(m + 1) * P],
                                    rhs=ct[n][k][:],
                                    start=(k == 0), stop=(k == 7))
                            ot = opool.tile([P, 4 * P], dt.bfloat16, tag="ot",
                                            name=f"ot{n}_{m}")
                            nc.scalar.activation(
                                out=ot[:], in_=pm[:],
                                func=mybir.ActivationFunctionType.Relu)
                            nc.sync.dma_start(
                                out=out_d[m * P:(m + 1) * P,
                                          n * 4 * P:(n + 1) * 4 * P],
                                in_=ot[:])

                assert idx_off == idx_tot

    nc.compile()
    return nc


def _classify(r):
    return np.searchsorted(np.asarray(CLS_BASE[1:]), r, side="right")


def _wrap_idxs(idx, pad_to):
    idx = np.asarray(idx, dtype=np.int64)
    n = len(idx)
    assert n <= pad_to, (n, pad_to)
    idx = np.concatenate([idx, np.zeros(pad_to - n, np.int64)])
    assert idx.max() <= 32767 and idx.min() >= 0
    wrapped = idx.astype(np.int16).reshape(pad_to // 16, 16).T
    return np.tile(wrapped, (8, 1))


def plan_sections(nodes, neigh_idx):
    """Neighbor sections: max-over-cores per-(tile,class) counts; self
    sections: max-over-cores per-(tile-pair,class). All 128-aligned."""
    nodes = np.asarray(nodes).astype(np.int64)
    neigh_idx = np.asarray(neigh_idx).astype(np.int64)
    sec_n = [[0] * 4 for _ in range(NT)]
    sec_s = [[0] * 4 for _ in range(NHC)]
    for core in range(NCORES):
        nd = nodes[core * BC:(core + 1) * BC]
        ng = neigh_idx[core * BC:(core + 1) * BC]
        cs = _classify(nd)
        cn = _classify(ng)
        for t in range(NT):
            lo = t * P
            for c in range(4):
                n_n = int((cn[lo:lo + P] == c).sum())
                sec_n[t][c] = max(sec_n[t][c], n_n)
        for hc in range(NHC):
            lo = 2 * hc * P
            for c in range(4):
                n_s = int((cs[lo:lo + 2 * P] == c).sum())
                sec_s[hc][c] = max(sec_s[hc][c], n_s)
    r128 = lambda x: (x + P - 1) // P * P
    sec_n = [[r128(sec_n[t][c]) for c in range(4)] for t in range(NT)]
    sec_s = [[r128(sec_s[hc][c]) for c in range(4)] for hc in range(NHC)]
    for t in range(NT):
        for c in range(4):
            assert sec_n[t][c] <= CAP_N[c], (t, c, sec_n[t][c])
    return sec_n, sec_s


def prep_core(nodes_c, neigh_c, sec_n, sec_s):
    cls_self = _classify(nodes_c)
    cls_neigh = _classify(neigh_c)
    ch_n = [[(sec_n[t][c] + P - 1) // P for c in range(4)] for t in range(NT)]
    max_ch = max(max(r) for r in ch_n)
    max_sch = max(sec_s[hc][c] // P for hc in range(NHC) for c in range(4))
    idc_per_t = 4 * max_sch + 4 * max_ch

    idx_all = []
    ids = np.full((P, NT * idc_per_t), 255.0, np.float32)

    # self streams: per class, per tile-pair section [t0 rows | t1 rows | pad]
    for c in range(4):
        for hc in range(NHC):
            t0 = 2 * hc
            secpos = []   # (tile, node_id_within_tile) per position
            rows = []
            for t in (t0, t0 + 1):
                lo = t * P
                nd = nodes_c[lo:lo + P]
                jj = np.nonzero(cls_self[lo:lo + P] == c)[0]
                o = np.argsort(nd[jj], kind="stable")
                jj = jj[o]
                rows.extend((nd[jj] - CLS_BASE[c]).tolist())
                secpos.extend((t, int(j)) for j in jj)
            sec = sec_s[hc][c]
            assert len(rows) <= sec, (hc, c, len(rows))
            rows.extend([0] * (sec - len(rows)))
            idx_all.append(np.asarray(rows, np.int64))
            # per-tile masked id planes over this section's chunks
            for t in (t0, t0 + 1):
                idbase = t * idc_per_t
                q = np.full(sec, 255.0, np.float32)
                for pos, (tt, j) in enumerate(secpos):
                    if tt == t:
                        q[pos] = j
                ids[:, idbase + c * max_sch:
                    idbase + c * max_sch + sec // P] =                     q.reshape(sec // P, P).T

    # neighbor streams per (hc, class): [t0 section, t1 section]
    for hc in range(NHC):
        for c in range(4):
            for t in (2 * hc, 2 * hc + 1):
                lo = t * P
                ng = neigh_c[lo:lo + P]
                jn, sn = np.nonzero(cls_neigh[lo:lo + P] == c)
                o = np.argsort(ng[jn, sn], kind="stable")
                jn, sn = jn[o], sn[o]
                n_n = len(jn)
                assert n_n <= sec_n[t][c], (t, c, n_n)
                rows = np.concatenate([
                    ng[jn, sn] - CLS_BASE[c],
                    np.zeros(sec_n[t][c] - n_n, np.int64),
                ])
                idx_all.append(rows)
                idbase = t * idc_per_t
                q = np.full(sec_n[t][c], 255.0, np.float32)
                q[:n_n] = jn
                ids[:, idbase + 4 * max_sch + c * max_ch:
                    idbase + 4 * max_sch + c * max_ch + ch_n[t][c]] = \
                    q.reshape(ch_n[t][c], P).T
    idx_flat = np.concatenate(idx_all)
    idx_arr = np.ascontiguousarray(_wrap_idxs(idx_flat, len(idx_flat)))
    return idx_arr, ids.astype(ml_dtypes.bfloat16)


def make_in_maps(features, weight, nodes, neigh_idx, sec_n, sec_s):
    from concourse import mybir

    features = np.asarray(features, dtype=np.float32)
    weight = np.asarray(weight, dtype=np.float32)
    nodes = np.asarray(nodes).astype(np.int64)
    neigh_idx = np.asarray(neigh_idx).astype(np.int64)

    feat_bf16 = features.astype(ml_dtypes.bfloat16)
    fp8_np = mybir.dt.np(mybir.dt.float8e4)
    feat_fp8 = features.astype(fp8_np)
    wt = weight.T.copy()
    wt[FEAT:, :] *= (1.0 / NSAMP)
    wt_bf16 = np.ascontiguousarray(wt.astype(ml_dtypes.bfloat16))

    in_maps = []
    for c in range(NCORES):
        nd = nodes[c * BC:(c + 1) * BC]
        ng = neigh_idx[c * BC:(c + 1) * BC]
        idx_arr, ids = prep_core(nd, ng, sec_n, sec_s)
        in_maps.append({
            "feat": feat_bf16,
            "featq": feat_fp8,
            "w_t": wt_bf16,
            "idx": idx_arr,
            "ids": ids,
        })
    return in_maps


def get_nc(sec_n, sec_s, reps=1):
    key = (tuple(map(tuple, sec_n)), tuple(map(tuple, sec_s)), reps)
    if key not in _CACHE:
        _CACHE[key] = build_nc(sec_n, sec_s, reps=reps)
    return _CACHE[key]


def kernel(features, weight, nodes, neigh_idx):
    from concourse import bass_utils

    sec_n, sec_s = plan_sections(nodes, neigh_idx)
    nc = get_nc(sec_n, sec_s)
    in_maps = make_in_maps(features, weight, nodes, neigh_idx, sec_n, sec_s)
    res = bass_utils.run_bass_kernel_spmd(
        nc, in_maps, core_ids=list(range(NCORES)), trace=False)
    out = np.concatenate(
        [np.asarray(r["out"]).astype(np.float32) for r in res.results],
        axis=1)
    return out
